# revision 1
# baseline (speedup 1.0000x reference)
"""Fused attention block (nn_Attention_27865747817251) on 8 trn2 NeuronCores.

Reference math (per batch b):
  y = x @ w_qkv + b_qkv                      # (L, 3D), D=2048, L=2048
  raw reshape (L, 3D) -> (3, NH, L, HD)      # NH=16, HD=128
  => per-head Q/K/V blocks are CONTIGUOUS ranges of y.flatten():
     q_h = flat[(0*NH+h)*L*HD : ...], k_h = flat[(NH+h)*L*HD : ...], v_h = ...
  A = softmax((K_h @ Q_h^T)/sqrt(HD), axis=-1);  out_h = A @ V_h
  out_bld[b, i, h*HD:(h+1)*HD] = out_h[i, :];  final = out_bld @ w_out + b_out

Sharding (8 cores):
  Launch A: qkv projection as y^T chunks, 2D grid: core k = (batch k//4,
    column-group k%4 of 12 chunks each). DMAs are strip-ordered so the PE
    starts after the first strips land instead of after the full input.
  Host: reassembles flat y, slices per-head Q^T/K^T/V.
  Launch B: core k = (batch, head-group g): attention for heads 4g..4g+3
    (S^T formulation; softmax sum via ones-matmul; A@V accumulated in PSUM)
    + row-parallel output projection -> partial (L, D) sums.
  Host: sums the 4 partials per batch, adds b_out.
"""

from contextlib import ExitStack

import numpy as np
import ml_dtypes

import concourse.bass as bass
from concourse import bacc
import concourse.mybir as mybir
import concourse.tile as tile
from concourse.bass_utils import run_bass_kernel_spmd

B, L, D = 2, 2048, 2048
NH, HD = 16, 128
D3 = 3 * D                      # 6144
NCHUNK = D3 // 128              # 48 column chunks of y
CPC = NCHUNK // 4               # 12 chunks per core (launch A, 4 col groups)
KT = D // 128                   # 16 k-subtiles
SCALE = 1.0 / float(np.sqrt(HD))

MM_DT = mybir.dt.bfloat16       # matmul operand dtype
NP_DT = ml_dtypes.bfloat16

_CACHE = {}


def _build_launch_a(reps=1):
    """Core k=(b, cg): y^T chunks [CPC, 128, L] = (w_qkv col-slice)^T @ x_b^T."""
    nc = bacc.Bacc()
    wq = nc.dram_tensor("wq", [128, KT, CPC * 128], MM_DT, kind="ExternalInput")
    xt = nc.dram_tensor("xt", [128, KT, L], MM_DT, kind="ExternalInput")
    yt = nc.dram_tensor("yt", [CPC, 128, L], MM_DT, kind="ExternalOutput")

    with tile.TileContext(nc) as tc, ExitStack() as ctx:
        singles = ctx.enter_context(tc.tile_pool(name="singles", bufs=1))
        outs = ctx.enter_context(tc.tile_pool(name="outs", bufs=4))
        psum = ctx.enter_context(tc.tile_pool(name="psum", bufs=4, space="PSUM"))

        for _rep in range(reps):
            wq_sb = singles.tile([128, KT, CPC * 128], MM_DT, tag="wq")
            xt_sb = singles.tile([128, KT, L], MM_DT, tag="xt")
            # strip-ordered loads: the first output tile only needs strip 0
            # of each, so PE work starts a few us in instead of after 14MB
            NRB = L // 512
            # interleave weight/activation strips so wq[cb] lands before the
            # PE's cb-major sweep reaches it (avoids mid-kernel DMA starvation)
            order = [("w", 0), ("x", 0), ("x", 1), ("w", 1), ("x", 2),
                     ("w", 2), ("x", 3)] + [("w", c) for c in range(3, CPC)]
            for kind, idx in order:
                if kind == "x":
                    nc.sync.dma_start(xt_sb[:, :, idx * 512:(idx + 1) * 512],
                                      xt[:, :, idx * 512:(idx + 1) * 512])
                else:
                    nc.sync.dma_start(wq_sb[:, :, idx * 128:(idx + 1) * 128],
                                      wq[:, :, idx * 128:(idx + 1) * 128])
            for cb in range(CPC):
                for rb in range(NRB):
                    pt = psum.tile([128, 512], mybir.dt.float32, tag="p")
                    for kt in range(KT):
                        nc.tensor.matmul(
                            pt[:],
                            wq_sb[:, kt, cb * 128:(cb + 1) * 128],
                            xt_sb[:, kt, rb * 512:(rb + 1) * 512],
                            start=(kt == 0),
                            stop=(kt == KT - 1),
                        )
                    ot = outs.tile([128, 512], MM_DT, tag="o")
                    nc.scalar.copy(ot[:], pt[:])
                    nc.sync.dma_start(yt[cb, :, rb * 512:(rb + 1) * 512], ot[:])
    nc.compile()
    return nc


def _build_launch_b(reps=1):
    """Core (b,g): attention for 4 heads + row-parallel out-proj partial."""
    HPC = 4                     # heads per core
    nc = bacc.Bacc()
    qt = nc.dram_tensor("qt", [128, HPC, L], MM_DT, kind="ExternalInput")
    kt_ = nc.dram_tensor("kt", [128, HPC, L], MM_DT, kind="ExternalInput")
    v = nc.dram_tensor("v", [128, HPC, L // 128, HD], MM_DT, kind="ExternalInput")
    wo = nc.dram_tensor("wo", [128, HPC, D], MM_DT, kind="ExternalInput")
    fp = nc.dram_tensor("fp", [L, D], mybir.dt.float32, kind="ExternalOutput")

    with tile.TileContext(nc) as tc, ExitStack() as ctx:
        singles = ctx.enter_context(tc.tile_pool(name="singles", bufs=1))
        pts = ctx.enter_context(tc.tile_pool(name="pts", bufs=6))
        norm = ctx.enter_context(tc.tile_pool(name="norm", bufs=2))
        fouts = ctx.enter_context(tc.tile_pool(name="fouts", bufs=6))

        for _rep in range(reps):
            qt_sb = singles.tile([128, HPC, L], MM_DT, tag="qt")
            kt_sb = singles.tile([128, HPC, L], MM_DT, tag="kt")
            v_sb = singles.tile([128, HPC, L // 128, HD], MM_DT, tag="v")
            wo_sb = singles.tile([128, HPC, D], MM_DT, tag="wo")
            ones_sb = singles.tile([128, 128], MM_DT, tag="ones")
            outT_sb = singles.tile([128, HPC, L], MM_DT, tag="outT")
            # per-head chunked loads so head 0's attention starts early
            for hh in range(HPC):
                nc.sync.dma_start(qt_sb[:, hh, :], qt[:, hh, :])
                nc.sync.dma_start(kt_sb[:, hh, :], kt_[:, hh, :])
                nc.sync.dma_start(v_sb[:, hh, :, :], v[:, hh, :, :])
            nc.sync.dma_start(wo_sb[:], wo[:])
            nc.vector.memset(ones_sb[:], 1.0)

            NIB = 4             # i blocks of 512 (K rows = output tokens)
            NJB = L // 128      # 16 j blocks (softmax dim)
            with ExitStack() as attn_ctx:
                psx = attn_ctx.enter_context(
                    tc.tile_pool(name="psx", bufs=4, space="PSUM"))
                pss = attn_ctx.enter_context(
                    tc.tile_pool(name="pss", bufs=2, space="PSUM"))
                pso = attn_ctx.enter_context(
                    tc.tile_pool(name="pso", bufs=2, space="PSUM"))
                for ib in range(NIB):
                    for hh in range(HPC):
                        ps_s = pss.tile([128, 512], mybir.dt.float32, tag="s")
                        ps_o = pso.tile([128, 512], mybir.dt.float32, tag="o")
                        for jb in range(NJB):
                            ps_x = psx.tile([128, 512], mybir.dt.float32,
                                            tag="x")
                            nc.tensor.matmul(
                                ps_x[:],
                                qt_sb[:, hh, jb * 128:(jb + 1) * 128],
                                kt_sb[:, hh, ib * 512:(ib + 1) * 512],
                                start=True, stop=True,
                            )
                            pT = pts.tile([128, 512], MM_DT, tag="pT")
                            nc.scalar.activation(
                                pT[:], ps_x[:],
                                mybir.ActivationFunctionType.Exp,
                                scale=SCALE,
                            )
                            nc.tensor.matmul(
                                ps_s[:], ones_sb[:], pT[:],
                                start=(jb == 0), stop=(jb == NJB - 1))
                            nc.tensor.matmul(
                                ps_o[:], v_sb[:, hh, jb, :], pT[:],
                                start=(jb == 0), stop=(jb == NJB - 1))
                        recip = norm.tile([128, 512], mybir.dt.float32,
                                          tag="r")
                        nc.vector.reciprocal(recip[:], ps_s[:])
                        nc.vector.tensor_mul(
                            out=outT_sb[:, hh, ib * 512:(ib + 1) * 512],
                            in0=ps_o[:], in1=recip[:],
                        )
            # out-proj phase (separate PSUM scope — banks reused)
            with tc.tile_pool(name="psf", bufs=6, space="PSUM") as psf:
                for rb in range(L // 128):
                    for cb in range(4):
                        ps_f = psf.tile([128, 512], mybir.dt.float32, tag="f")
                        for hh in range(HPC):
                            nc.tensor.matmul(
                                ps_f[:],
                                outT_sb[:, hh, rb * 128:(rb + 1) * 128],
                                wo_sb[:, hh, cb * 512:(cb + 1) * 512],
                                start=(hh == 0), stop=(hh == 3),
                            )
                        fo = fouts.tile([128, 512], mybir.dt.float32,
                                        tag="fo")
                        if cb % 2 == 0:
                            nc.vector.tensor_copy(fo[:], ps_f[:])
                        else:
                            nc.scalar.copy(fo[:], ps_f[:])
                        nc.sync.dma_start(
                            fp[rb * 128:(rb + 1) * 128,
                               cb * 512:(cb + 1) * 512],
                            fo[:],
                        )
    nc.compile()
    return nc


def _get(name, reps=1):
    key = (name, reps)
    if key not in _CACHE:
        _CACHE[key] = (_build_launch_a(reps) if name == "a"
                       else _build_launch_b(reps))
    return _CACHE[key]


def _prep_a(x, w_qkv):
    in_a = []
    for k in range(8):
        b, cg = k // 4, k % 4
        wsl = w_qkv[:, cg * CPC * 128:(cg + 1) * CPC * 128]
        wq_h = np.ascontiguousarray(
            wsl.reshape(KT, 128, CPC * 128).transpose(1, 0, 2)).astype(NP_DT)
        xt_h = np.ascontiguousarray(
            x[b].T.reshape(KT, 128, L).transpose(1, 0, 2)).astype(NP_DT)
        in_a.append({"wq": wq_h, "xt": xt_h})
    return in_a


def _prep_b(ya_list, b_qkv, w_out):
    """ya_list: 8 arrays [CPC, 128, L]; returns per-core launch-B inputs."""
    sec = L * HD
    in_b = []
    for b in range(B):
        yb = np.concatenate([ya_list[b * 4 + cg] for cg in range(4)], axis=0)
        if b_qkv.any():
            yb = (yb.astype(np.float32)
                  + b_qkv.reshape(NCHUNK, 128)[:, :, None]).astype(NP_DT)
        flat = np.ascontiguousarray(yb.transpose(2, 0, 1)).reshape(-1)
        for g in range(4):
            qts, kts, vs = [], [], []
            for hh in range(4):
                h = 4 * g + hh
                qh = flat[h * sec:(h + 1) * sec].reshape(L, HD)
                kh = flat[(NH + h) * sec:(NH + h + 1) * sec].reshape(L, HD)
                vh = flat[(2 * NH + h) * sec:(2 * NH + h + 1) * sec].reshape(L, HD)
                qts.append(qh.T)
                kts.append(kh.T)
                vs.append(vh.reshape(L // 128, 128, HD).transpose(1, 0, 2))
            wsl = w_out[g * 512:(g + 1) * 512, :]
            wo_h = np.ascontiguousarray(
                wsl.reshape(4, 128, D).transpose(1, 0, 2)).astype(NP_DT)
            in_b.append({
                "qt": np.ascontiguousarray(np.stack(qts, axis=1)),
                "kt": np.ascontiguousarray(np.stack(kts, axis=1)),
                "v": np.ascontiguousarray(np.stack(vs, axis=1)),
                "wo": wo_h,
            })
    return in_b


def kernel(x, w_qkv, b_qkv, w_out, b_out, _timing=None):
    x = np.asarray(x, dtype=np.float32)
    w_qkv = np.asarray(w_qkv, dtype=np.float32)
    b_qkv = np.asarray(b_qkv, dtype=np.float32)
    w_out = np.asarray(w_out, dtype=np.float32)
    b_out = np.asarray(b_out, dtype=np.float32)
    cores = list(range(8))

    in_a = _prep_a(x, w_qkv)
    res_a = run_bass_kernel_spmd(_get("a"), in_a, cores)
    ya = [np.asarray(res_a.results[k]["yt"]) for k in range(8)]

    in_b = _prep_b(ya, b_qkv, w_out)
    res_b = run_bass_kernel_spmd(_get("b"), in_b, cores)

    out = np.empty((B, L, D), dtype=np.float32)
    for b in range(B):
        acc = np.zeros((L, D), dtype=np.float32)
        for g in range(4):
            acc += np.asarray(res_b.results[b * 4 + g]["fp"])
        out[b] = acc + b_out[None, :]
    return out



# revision 6
# speedup vs baseline: 1.1543x; 1.1543x over previous
"""Fused attention block (nn_Attention_27865747817251) on 8 trn2 NeuronCores.

Reference math (per batch b):
  y = x @ w_qkv + b_qkv                      # (L, 3D), D=2048, L=2048
  raw reshape (L, 3D) -> (3, NH, L, HD)      # NH=16, HD=128, NO transpose
  => per-head Q/K/V are CONTIGUOUS ranges of y.flatten():
     q_h = flat[(0*NH+h)*L*HD : ...], k_h = flat[(NH+h)*L*HD : ...], ...
  A = softmax((K_h @ Q_h^T)/sqrt(HD), axis=-1);  out_h = A @ V_h
  out_bld[b, i, h*HD:(h+1)*HD] = out_h[i, :];  final = out_bld @ w_out + b_out

Two SPMD launches on 8 cores (the scramble between them runs on host):

Launch A - QKV projection, core k = (batch k//4, column-group k%4):
  y^T chunks [12, 128, L] = (w_qkv col-slice)^T @ x_b^T, in fp8e4
  DoubleRow (256-deep reduction tiles) with a hi+lo residual split at
  matched scales:
      X1=Q(x), X2=Q(x-X1), W1=Q(32w), W2=Q(32w-W1)
      G = X1@W1 + X1@W2 + X2@W1  (one PSUM group; ~bf16 accuracy)
      y' = 16*G = 512*y  (compensated downstream: exp scale /512^2 and
      w_out/512).  0.75x the PE cycles of bf16 at 4x DoubleRow rate.

Launch B - attention + out-proj, core k = (batch, head-group of 4):
  S^T formulation; softmax denominators OFF the PE: exp writes a
  [128, 512, 16] super-tile (j-tile innermost), DVE tensor_reduce sums
  the 16 j-tiles, gpsimd partition_all_reduce sums the partitions
  (replaces the baseline's ones-matmul: -131k PE cycles/core).
  Out-proj (row-parallel partial) interleaved one i-block behind
  attention so the PE stays fed while Act runs exp.  Host sums the 4
  partials per batch and adds b_out.
"""

from contextlib import ExitStack

import numpy as np
import ml_dtypes

import concourse.bass as bass
from concourse import bacc
import concourse.mybir as mybir
import concourse.tile as tile
from concourse.bass_utils import run_bass_kernel_spmd
from concourse.bass_isa import ReduceOp
from concourse.alu_op_type import AluOpType

B, L, D = 2, 2048, 2048
NH, HD = 16, 128
HPC = 4                         # heads per core (launch B)
CPC = 12                        # y^T chunks per core (launch A)
KT = D // 128                   # 16 contraction k-tiles
SCALE = 1.0 / float(np.sqrt(HD))
YS = 512.0                      # y' = YS * y leaves launch A
CEXP = SCALE / (YS * YS)        # exp scale on raw score PSUM

F8 = mybir.dt.float8e4
BF = mybir.dt.bfloat16
F32 = mybir.dt.float32
NP_F8 = ml_dtypes.float8_e4m3fn
NP_BF = ml_dtypes.bfloat16
DR = mybir.MatmulPerfMode.DoubleRow

_CACHE = {}


def _build_a():
    """Core k=(b, cg): y'^T chunks [CPC, 128, L] in bf16, y' = 512*y."""
    nc = bacc.Bacc()
    x1 = nc.dram_tensor("x1", [128, KT, L], F8, kind="ExternalInput")
    x2 = nc.dram_tensor("x2", [128, KT, L], F8, kind="ExternalInput")
    w1 = nc.dram_tensor("w1", [128, KT, CPC * 128], F8, kind="ExternalInput")
    w2 = nc.dram_tensor("w2", [128, KT, CPC * 128], F8, kind="ExternalInput")
    yt = nc.dram_tensor("yt", [CPC, 128, L], BF, kind="ExternalOutput")

    with tile.TileContext(nc) as tc, ExitStack() as ctx:
        wp = ctx.enter_context(tc.tile_pool(name="wp", bufs=1))
        xp = ctx.enter_context(tc.tile_pool(name="xp", bufs=2))
        outs = ctx.enter_context(tc.tile_pool(name="outs", bufs=6))
        psg = ctx.enter_context(tc.tile_pool(name="psg", bufs=4, space="PSUM"))

        w1_sb = wp.tile([128, KT, CPC * 128], F8, tag="w1")
        w2_sb = wp.tile([128, KT, CPC * 128], F8, tag="w2")

        nc.sync.dma_start(w1_sb[:], w1[:])
        xs = []
        x1s0 = xp.tile([128, KT, 512], F8, tag="x1s", name="x1s0")
        x2s0 = xp.tile([128, KT, 512], F8, tag="x2s", name="x2s0")
        nc.sync.dma_start(x1s0[:], x1[:, :, 0:512])
        nc.sync.dma_start(x2s0[:], x2[:, :, 0:512])
        nc.sync.dma_start(w2_sb[:], w2[:])

        for rb in range(4):
            s0, s1 = rb * 512, (rb + 1) * 512
            if rb == 0:
                x1s, x2s = x1s0, x2s0
            else:
                x1s = xp.tile([128, KT, 512], F8, tag="x1s", name=f"x1s{rb}")
                x2s = xp.tile([128, KT, 512], F8, tag="x2s", name=f"x2s{rb}")
                nc.sync.dma_start(x1s[:], x1[:, :, s0:s1])
                nc.sync.dma_start(x2s[:], x2[:, :, s0:s1])
            for c in range(CPC):
                g = psg.tile([128, 512], F32, tag="g")
                cs = slice(c * 128, (c + 1) * 128)
                for term, (wsb, xsb) in enumerate(
                        ((w1_sb, x1s), (w2_sb, x1s), (w1_sb, x2s))):
                    for kp in range(8):
                        nc.tensor.matmul(
                            g[:],
                            wsb[:, 2 * kp:2 * kp + 2, cs],
                            xsb[:, 2 * kp:2 * kp + 2, :],
                            start=(term == 0 and kp == 0),
                            stop=(term == 2 and kp == 7),
                            perf_mode=DR,
                        )
                ot = outs.tile([128, 512], BF, tag="o")
                nc.scalar.mul(ot[:], g[:], 16.0)
                nc.sync.dma_start(yt[c, :, s0:s1], ot[:])
    nc.compile()
    return nc


def _build_b():
    """Core (b, g): attention for 4 heads + row-parallel out-proj partial."""
    nc = bacc.Bacc()
    qt = nc.dram_tensor("qt", [128, HPC, L], BF, kind="ExternalInput")
    kt = nc.dram_tensor("kt", [128, HPC, L], BF, kind="ExternalInput")
    v = nc.dram_tensor("v", [128, HPC, L // 128, HD], BF, kind="ExternalInput")
    wo = nc.dram_tensor("wo", [128, HPC, D], BF, kind="ExternalInput")
    fp = nc.dram_tensor("fp", [L, D], F32, kind="ExternalOutput")

    with tile.TileContext(nc) as tc, ExitStack() as ctx:
        singles = ctx.enter_context(tc.tile_pool(name="singles", bufs=1))
        pts = ctx.enter_context(tc.tile_pool(name="pts", bufs=2))
        nrm = ctx.enter_context(tc.tile_pool(name="nrm", bufs=2))
        fout = ctx.enter_context(tc.tile_pool(name="fout", bufs=4))
        psx = ctx.enter_context(tc.tile_pool(name="psx", bufs=2, space="PSUM"))
        pso = ctx.enter_context(tc.tile_pool(name="pso", bufs=2, space="PSUM"))
        psf = ctx.enter_context(tc.tile_pool(name="psf", bufs=2, space="PSUM"))

        qt_sb = singles.tile([128, HPC, L], BF, tag="qt")
        kt_sb = singles.tile([128, HPC, L], BF, tag="kt")
        v_sb = singles.tile([128, HPC, L // 128, HD], BF, tag="v")
        wo_sb = singles.tile([128, HPC, D], BF, tag="wo")
        outT = singles.tile([128, HPC, L], BF, tag="outT")

        # per-head loads so head 0's attention starts early
        for hh in range(HPC):
            nc.sync.dma_start(qt_sb[:, hh, :], qt[:, hh, :])
            nc.sync.dma_start(kt_sb[:, hh, :], kt[:, hh, :])
            nc.sync.dma_start(v_sb[:, hh, :, :], v[:, hh, :, :])
        nc.sync.dma_start(wo_sb[:], wo[:])

        def outproj_piece(ib, rb):
            r0 = (ib * 4 + rb) * 128
            for cb in range(4):
                pf = psf.tile([128, 512], F32, tag="f")
                c0 = cb * 512
                for hh in range(HPC):
                    nc.tensor.matmul(
                        pf[:],
                        outT[:, hh, r0:r0 + 128],
                        wo_sb[:, hh, c0:c0 + 512],
                        start=(hh == 0), stop=(hh == HPC - 1),
                    )
                fo = fout.tile([128, 512], F32, tag="fo")
                nc.vector.tensor_copy(fo[:], pf[:])
                nc.sync.dma_start(fp[r0:r0 + 128, c0:c0 + 512], fo[:])

        for ib in range(4):
            i0, i1 = ib * 512, (ib + 1) * 512
            for hh in range(HPC):
                ptile = pts.tile([128, 512, 16], BF, tag="pt")
                ps_o = pso.tile([128, 512], F32, tag="o")
                for jb2 in range(8):
                    ps_x = psx.tile([128, 2, 512], F32, tag="x")
                    for t in range(2):
                        jb = 2 * jb2 + t
                        nc.tensor.matmul(
                            ps_x[:, t, :],
                            qt_sb[:, hh, jb * 128:(jb + 1) * 128],
                            kt_sb[:, hh, i0:i1],
                            start=True, stop=True,
                        )
                    nc.scalar.activation(
                        ptile[:, :, 2 * jb2:2 * jb2 + 2],
                        ps_x.rearrange("p t i -> p i t"),
                        mybir.ActivationFunctionType.Exp, scale=CEXP)
                for jb in range(16):
                    nc.tensor.matmul(
                        ps_o[:],
                        v_sb[:, hh, jb, :],
                        ptile[:, :, jb],
                        start=(jb == 0), stop=(jb == 15),
                    )
                sums = nrm.tile([128, 512], BF, tag="s")
                with nc.allow_low_precision(
                        reason="softmax denom j-tile partials; "
                        "fp32 partition allreduce follows"):
                    nc.vector.tensor_reduce(
                        sums[:], ptile[:], mybir.AxisListType.X, AluOpType.add)
                sumf = nrm.tile([128, 512], F32, tag="sf")
                nc.gpsimd.partition_all_reduce(
                    sumf[:], sums[:], 128, ReduceOp.add)
                recip = nrm.tile([128, 512], F32, tag="r")
                nc.vector.reciprocal(recip[:], sumf[:])
                nc.vector.tensor_mul(
                    out=outT[:, hh, i0:i1], in0=ps_o[:], in1=recip[:])
                if ib > 0:
                    outproj_piece(ib - 1, hh)
        for rb in range(4):
            outproj_piece(3, rb)
    nc.compile()
    return nc


def _get(name):
    if name not in _CACHE:
        _CACHE[name] = _build_a() if name == "a" else _build_b()
    return _CACHE[name]


def _q8(a):
    return a.astype(NP_F8)


def _prep_a(x, w_qkv):
    """Per-core launch-A inputs; core k = (b, cg)."""
    ins = []
    xq = {}
    for b in range(B):
        xt = np.ascontiguousarray(
            x[b].T.reshape(KT, 128, L).transpose(1, 0, 2))
        x1 = _q8(xt)
        x2 = _q8(xt - x1.astype(np.float32))
        xq[b] = (x1, x2)
    for k in range(8):
        b, cg = k // 4, k % 4
        wsl = w_qkv[:, cg * CPC * 128:(cg + 1) * CPC * 128] * 32.0
        wt = np.ascontiguousarray(
            wsl.reshape(KT, 128, CPC * 128).transpose(1, 0, 2))
        w1 = _q8(wt)
        w2 = _q8(wt - w1.astype(np.float32))
        ins.append({"x1": xq[b][0], "x2": xq[b][1], "w1": w1, "w2": w2})
    return ins


def _prep_b(ya_list, b_qkv, w_out):
    """ya_list: 8 arrays [CPC, 128, L] (y' = 512*y); per-core B inputs."""
    sec = L * HD
    ins = []
    for b in range(B):
        yb = np.concatenate([ya_list[b * 4 + cg] for cg in range(4)], axis=0)
        if b_qkv.any():
            yb = (yb.astype(np.float32)
                  + YS * b_qkv.reshape(48, 128)[:, :, None]).astype(NP_BF)
        flat = np.ascontiguousarray(yb.transpose(2, 0, 1)).reshape(-1)
        for g in range(4):
            qts, kts, vs = [], [], []
            for hh in range(HPC):
                h = HPC * g + hh
                qh = flat[h * sec:(h + 1) * sec].reshape(L, HD)
                kh = flat[(NH + h) * sec:(NH + h + 1) * sec].reshape(L, HD)
                vh = flat[(2 * NH + h) * sec:(2 * NH + h + 1) * sec].reshape(
                    L, HD)
                qts.append(qh.T)
                kts.append(kh.T)
                vs.append(vh.reshape(L // 128, 128, HD).transpose(1, 0, 2))
            wsl = w_out[512 * g:512 * (g + 1), :]
            wo_h = np.ascontiguousarray(
                wsl.reshape(HPC, 128, D).transpose(1, 0, 2) / YS).astype(NP_BF)
            ins.append({
                "qt": np.ascontiguousarray(np.stack(qts, axis=1)),
                "kt": np.ascontiguousarray(np.stack(kts, axis=1)),
                "v": np.ascontiguousarray(np.stack(vs, axis=1)),
                "wo": wo_h,
            })
    return ins


def kernel(x, w_qkv, b_qkv, w_out, b_out, _timing=None):
    x = np.asarray(x, dtype=np.float32)
    w_qkv = np.asarray(w_qkv, dtype=np.float32)
    b_qkv = np.asarray(b_qkv, dtype=np.float32)
    w_out = np.asarray(w_out, dtype=np.float32)
    b_out = np.asarray(b_out, dtype=np.float32)
    cores = list(range(8))

    res_a = run_bass_kernel_spmd(_get("a"), _prep_a(x, w_qkv), cores)
    ya = [np.asarray(res_a.results[k]["yt"]) for k in range(8)]

    res_b = run_bass_kernel_spmd(_get("b"), _prep_b(ya, b_qkv, w_out), cores)

    out = np.empty((B, L, D), dtype=np.float32)
    for b in range(B):
        acc = np.zeros((L, D), dtype=np.float32)
        for g in range(4):
            acc += np.asarray(res_b.results[b * 4 + g]["fp"])
        out[b] = acc + b_out[None, :]
    return out


# revision 7
# speedup vs baseline: 1.3060x; 1.1314x over previous
"""Fused attention block (nn_Attention_27865747817251) on 8 trn2 NeuronCores.

Reference math (per batch b):
  y = x @ w_qkv + b_qkv                      # (L, 3D), D=2048, L=2048
  raw reshape (L, 3D) -> (3, NH, L, HD)      # NH=16, HD=128, NO transpose
  => per-head Q/K/V are CONTIGUOUS ranges of y.flatten():
     q_h = flat[(0*NH+h)*L*HD : ...], k_h = flat[(NH+h)*L*HD : ...], ...
  A = softmax((K_h @ Q_h^T)/sqrt(HD), axis=-1);  out_h = A @ V_h
  out_bld[b, i, h*HD:(h+1)*HD] = out_h[i, :];  final = out_bld @ w_out + b_out

Two SPMD launches on 8 cores (the scramble between them runs on host):

Launch A - QKV projection, core k = (batch k//4, column-group k%4):
  y^T chunks [12, 128, L] = (w_qkv col-slice)^T @ x_b^T, in fp8e4
  DoubleRow (256-deep reduction tiles) with a hi+lo residual split at
  matched scales:
      X1=Q(x), X2=Q(x-X1), W1=Q(32w), W2=Q(32w-W1)
      G = X1@W1 + X1@W2 + X2@W1  (one PSUM group; ~bf16 accuracy)
      y' = 16*G = 512*y  (compensated downstream: exp scale /512^2 and
      w_out/512).  0.75x the PE cycles of bf16 at 4x DoubleRow rate.

Launch B - attention + out-proj, core k = (batch, head-group of 4):
  S^T formulation; softmax denominators OFF the PE: exp writes a
  [128, 512, 16] super-tile (j-tile innermost), DVE tensor_reduce sums
  the 16 j-tiles, gpsimd partition_all_reduce sums the partitions
  (replaces the baseline's ones-matmul: -131k PE cycles/core).
  Out-proj (row-parallel partial) interleaved one i-block behind
  attention so the PE stays fed while Act runs exp.  Host sums the 4
  partials per batch and adds b_out.
"""

from contextlib import ExitStack

import numpy as np
import ml_dtypes

import concourse.bass as bass
from concourse import bacc
import concourse.mybir as mybir
import concourse.tile as tile
from concourse.bass_utils import run_bass_kernel_spmd
from concourse.bass_isa import ReduceOp
from concourse.alu_op_type import AluOpType

B, L, D = 2, 2048, 2048
NH, HD = 16, 128
HPC = 4                         # heads per core (launch B)
CPC = 12                        # y^T chunks per core (launch A)
KT = D // 128                   # 16 contraction k-tiles
SCALE = 1.0 / float(np.sqrt(HD))
YS = 512.0                      # y' = YS * y leaves launch A
CEXP = SCALE / (YS * YS)        # exp scale on raw score PSUM

F8 = mybir.dt.float8e4
BF = mybir.dt.bfloat16
F32 = mybir.dt.float32
NP_F8 = ml_dtypes.float8_e4m3fn
NP_BF = ml_dtypes.bfloat16
DR = mybir.MatmulPerfMode.DoubleRow

_CACHE = {}


def _build_a():
    """Core k=(b, cg): y'^T chunks [CPC, 128, L] in bf16, y' = 512*y."""
    nc = bacc.Bacc()
    x1 = nc.dram_tensor("x1", [128, KT, L], F8, kind="ExternalInput")
    x2 = nc.dram_tensor("x2", [128, KT, L], F8, kind="ExternalInput")
    w1 = nc.dram_tensor("w1", [128, KT, CPC * 128], F8, kind="ExternalInput")
    w2 = nc.dram_tensor("w2", [128, KT, CPC * 128], F8, kind="ExternalInput")
    yt = nc.dram_tensor("yt", [CPC, 128, L], BF, kind="ExternalOutput")

    with tile.TileContext(nc) as tc, ExitStack() as ctx:
        wp = ctx.enter_context(tc.tile_pool(name="wp", bufs=1))
        xp = ctx.enter_context(tc.tile_pool(name="xp", bufs=2))
        outs = ctx.enter_context(tc.tile_pool(name="outs", bufs=6))
        psg = ctx.enter_context(tc.tile_pool(name="psg", bufs=4, space="PSUM"))

        w1_sb = wp.tile([128, KT, CPC * 128], F8, tag="w1")
        w2_sb = wp.tile([128, KT, CPC * 128], F8, tag="w2")

        # PE warmup during the DMA lead so real matmuls start at full clock
        wu = wp.tile([128, 512], BF, tag="wu")
        nc.vector.memset(wu[:], 0.0)
        for _ in range(28):
            pwu = psg.tile([128, 512], F32, tag="g")
            nc.tensor.matmul(pwu[:], wu[:, 0:128], wu[:], start=True,
                             stop=True)

        # piecewise strip-ordered loads: first group's deps first
        nc.sync.dma_start(w1_sb[:, :, 0:384], w1[:, :, 0:384])
        x1s0 = xp.tile([128, KT, 512], F8, tag="x1s", name="x1s0")
        x2s0 = xp.tile([128, KT, 512], F8, tag="x2s", name="x2s0")
        nc.sync.dma_start(x1s0[:], x1[:, :, 0:512])
        nc.sync.dma_start(x2s0[:], x2[:, :, 0:512])
        nc.sync.dma_start(w2_sb[:, :, 0:384], w2[:, :, 0:384])
        for pc in range(1, 4):
            ws = slice(pc * 384, (pc + 1) * 384)
            nc.sync.dma_start(w1_sb[:, :, ws], w1[:, :, ws])
            nc.sync.dma_start(w2_sb[:, :, ws], w2[:, :, ws])

        for rb in range(4):
            s0, s1 = rb * 512, (rb + 1) * 512
            if rb == 0:
                x1s, x2s = x1s0, x2s0
            else:
                x1s = xp.tile([128, KT, 512], F8, tag="x1s", name=f"x1s{rb}")
                x2s = xp.tile([128, KT, 512], F8, tag="x2s", name=f"x2s{rb}")
                nc.sync.dma_start(x1s[:], x1[:, :, s0:s1])
                nc.sync.dma_start(x2s[:], x2[:, :, s0:s1])
            for c in range(CPC):
                g = psg.tile([128, 512], F32, tag="g")
                cs = slice(c * 128, (c + 1) * 128)
                for term, (wsb, xsb) in enumerate(
                        ((w1_sb, x1s), (w1_sb, x2s), (w2_sb, x1s))):
                    for kp in range(8):
                        nc.tensor.matmul(
                            g[:],
                            wsb[:, 2 * kp:2 * kp + 2, cs],
                            xsb[:, 2 * kp:2 * kp + 2, :],
                            start=(term == 0 and kp == 0),
                            stop=(term == 2 and kp == 7),
                            perf_mode=DR,
                        )
                ot = outs.tile([128, 512], BF, tag="o")
                nc.scalar.mul(ot[:], g[:], 16.0)
                nc.sync.dma_start(yt[c, :, s0:s1], ot[:])
    nc.compile()
    return nc


def _build_b():
    """Core (b, g): attention for 4 heads + row-parallel out-proj partial."""
    nc = bacc.Bacc()
    qt = nc.dram_tensor("qt", [128, HPC, L], BF, kind="ExternalInput")
    kt = nc.dram_tensor("kt", [128, HPC, L], BF, kind="ExternalInput")
    v = nc.dram_tensor("v", [128, HPC, L // 128, HD], BF, kind="ExternalInput")
    wo = nc.dram_tensor("wo", [128, HPC, D], BF, kind="ExternalInput")
    fp = nc.dram_tensor("fp", [L, D], F32, kind="ExternalOutput")

    with tile.TileContext(nc) as tc, ExitStack() as ctx:
        singles = ctx.enter_context(tc.tile_pool(name="singles", bufs=1))
        pts = ctx.enter_context(tc.tile_pool(name="pts", bufs=2))
        nrm = ctx.enter_context(tc.tile_pool(name="nrm", bufs=2))
        fout = ctx.enter_context(tc.tile_pool(name="fout", bufs=4))
        psx = ctx.enter_context(tc.tile_pool(name="psx", bufs=2, space="PSUM"))
        pso = ctx.enter_context(tc.tile_pool(name="pso", bufs=2, space="PSUM"))
        psf = ctx.enter_context(tc.tile_pool(name="psf", bufs=2, space="PSUM"))

        qt_sb = singles.tile([128, HPC, L], BF, tag="qt")
        kt_sb = singles.tile([128, HPC, L], BF, tag="kt")
        v_sb = singles.tile([128, HPC, L // 128, HD], BF, tag="v")
        wo_sb = singles.tile([128, HPC, D], BF, tag="wo")
        outT = singles.tile([128, HPC, L], BF, tag="outT")

        # PE warmup during the DMA lead so real matmuls start at full clock
        wu = singles.tile([128, 512], BF, tag="wu")
        nc.vector.memset(wu[:], 0.0)
        for _ in range(20):
            pwu = psx.tile([128, 2, 512], F32, tag="x")
            nc.tensor.matmul(pwu[:, 0, :], wu[:, 0:128], wu[:], start=True,
                             stop=True)

        # per-head loads so head 0's attention starts early; kt sliced so
        # the first score matmul only waits for qt[h0] + kt[h0] slice 0
        for hh in range(HPC):
            nc.sync.dma_start(qt_sb[:, hh, :], qt[:, hh, :])
            for sl in range(4):
                nc.sync.dma_start(kt_sb[:, hh, sl * 512:(sl + 1) * 512],
                                  kt[:, hh, sl * 512:(sl + 1) * 512])
            nc.sync.dma_start(v_sb[:, hh, :, :], v[:, hh, :, :])
        nc.sync.dma_start(wo_sb[:], wo[:])

        def outproj_piece(ib, rb):
            r0 = (ib * 4 + rb) * 128
            for cb in range(4):
                pf = psf.tile([128, 512], F32, tag="f")
                c0 = cb * 512
                for hh in range(HPC):
                    nc.tensor.matmul(
                        pf[:],
                        outT[:, hh, r0:r0 + 128],
                        wo_sb[:, hh, c0:c0 + 512],
                        start=(hh == 0), stop=(hh == HPC - 1),
                    )
                fo = fout.tile([128, 512], F32, tag="fo")
                if cb == 3:
                    nc.scalar.copy(fo[:], pf[:])
                else:
                    nc.vector.tensor_copy(fo[:], pf[:])
                nc.sync.dma_start(fp[r0:r0 + 128, c0:c0 + 512], fo[:])

        for ib in range(4):
            i0, i1 = ib * 512, (ib + 1) * 512
            for hh in range(HPC):
                ptile = pts.tile([128, 16, 512], BF, tag="pt")
                ps_o = pso.tile([128, 512], F32, tag="o")
                sums = nrm.tile([128, 512], BF, tag="s")
                with nc.allow_low_precision(
                        reason="softmax denom j-tile partial sums in bf16; "
                        "fp32 partition allreduce follows"):
                    for jb2 in range(8):
                        ps_x = psx.tile([128, 2, 512], F32, tag="x")
                        for t in range(2):
                            jb = 2 * jb2 + t
                            nc.tensor.matmul(
                                ps_x[:, t, :],
                                qt_sb[:, hh, jb * 128:(jb + 1) * 128],
                                kt_sb[:, hh, i0:i1],
                                start=True, stop=True,
                            )
                        nc.scalar.activation(
                            ptile[:, 2 * jb2:2 * jb2 + 2, :], ps_x[:],
                            mybir.ActivationFunctionType.Exp, scale=CEXP)
                        # j-denominator accumulation rides the exp pipeline
                        if jb2 == 0:
                            nc.vector.tensor_add(
                                sums[:], ptile[:, 0, :], ptile[:, 1, :])
                        else:
                            for t in range(2):
                                nc.vector.tensor_add(
                                    sums[:], sums[:],
                                    ptile[:, 2 * jb2 + t, :])
                for jb in range(16):
                    nc.tensor.matmul(
                        ps_o[:],
                        v_sb[:, hh, jb, :],
                        ptile[:, jb, :],
                        start=(jb == 0), stop=(jb == 15),
                    )
                sumf = nrm.tile([128, 512], F32, tag="sf")
                nc.gpsimd.partition_all_reduce(
                    sumf[:], sums[:], 128, ReduceOp.add)
                recip = nrm.tile([128, 512], F32, tag="r")
                nc.vector.reciprocal(recip[:], sumf[:])
                nc.vector.tensor_mul(
                    out=outT[:, hh, i0:i1], in0=ps_o[:], in1=recip[:])
                if ib > 0:
                    outproj_piece(ib - 1, hh)
        for rb in range(4):
            outproj_piece(3, rb)
    nc.compile()
    return nc


def _get(name):
    if name not in _CACHE:
        _CACHE[name] = _build_a() if name == "a" else _build_b()
    return _CACHE[name]


def _q8(a):
    return a.astype(NP_F8)


def _prep_a(x, w_qkv):
    """Per-core launch-A inputs; core k = (b, cg)."""
    ins = []
    xq = {}
    for b in range(B):
        xt = np.ascontiguousarray(
            x[b].T.reshape(KT, 128, L).transpose(1, 0, 2))
        x1 = _q8(xt)
        x2 = _q8(xt - x1.astype(np.float32))
        xq[b] = (x1, x2)
    for k in range(8):
        b, cg = k // 4, k % 4
        wsl = w_qkv[:, cg * CPC * 128:(cg + 1) * CPC * 128] * 32.0
        wt = np.ascontiguousarray(
            wsl.reshape(KT, 128, CPC * 128).transpose(1, 0, 2))
        w1 = _q8(wt)
        w2 = _q8(wt - w1.astype(np.float32))
        ins.append({"x1": xq[b][0], "x2": xq[b][1], "w1": w1, "w2": w2})
    return ins


def _prep_b(ya_list, b_qkv, w_out):
    """ya_list: 8 arrays [CPC, 128, L] (y' = 512*y); per-core B inputs."""
    sec = L * HD
    ins = []
    for b in range(B):
        yb = np.concatenate([ya_list[b * 4 + cg] for cg in range(4)], axis=0)
        if b_qkv.any():
            yb = (yb.astype(np.float32)
                  + YS * b_qkv.reshape(48, 128)[:, :, None]).astype(NP_BF)
        flat = np.ascontiguousarray(yb.transpose(2, 0, 1)).reshape(-1)
        for g in range(4):
            qts, kts, vs = [], [], []
            for hh in range(HPC):
                h = HPC * g + hh
                qh = flat[h * sec:(h + 1) * sec].reshape(L, HD)
                kh = flat[(NH + h) * sec:(NH + h + 1) * sec].reshape(L, HD)
                vh = flat[(2 * NH + h) * sec:(2 * NH + h + 1) * sec].reshape(
                    L, HD)
                qts.append(qh.T)
                kts.append(kh.T)
                vs.append(vh.reshape(L // 128, 128, HD).transpose(1, 0, 2))
            wsl = w_out[512 * g:512 * (g + 1), :]
            wo_h = np.ascontiguousarray(
                wsl.reshape(HPC, 128, D).transpose(1, 0, 2) / YS).astype(NP_BF)
            ins.append({
                "qt": np.ascontiguousarray(np.stack(qts, axis=1)),
                "kt": np.ascontiguousarray(np.stack(kts, axis=1)),
                "v": np.ascontiguousarray(np.stack(vs, axis=1)),
                "wo": wo_h,
            })
    return ins


def kernel(x, w_qkv, b_qkv, w_out, b_out, _timing=None):
    x = np.asarray(x, dtype=np.float32)
    w_qkv = np.asarray(w_qkv, dtype=np.float32)
    b_qkv = np.asarray(b_qkv, dtype=np.float32)
    w_out = np.asarray(w_out, dtype=np.float32)
    b_out = np.asarray(b_out, dtype=np.float32)
    cores = list(range(8))

    res_a = run_bass_kernel_spmd(_get("a"), _prep_a(x, w_qkv), cores)
    ya = [np.asarray(res_a.results[k]["yt"]) for k in range(8)]

    res_b = run_bass_kernel_spmd(_get("b"), _prep_b(ya, b_qkv, w_out), cores)

    out = np.empty((B, L, D), dtype=np.float32)
    for b in range(B):
        acc = np.zeros((L, D), dtype=np.float32)
        for g in range(4):
            acc += np.asarray(res_b.results[b * 4 + g]["fp"])
        out[b] = acc + b_out[None, :]
    return out


# revision 9
# speedup vs baseline: 1.3553x; 1.0378x over previous
"""Fused attention block (nn_Attention_27865747817251) on 8 trn2 NeuronCores.

Reference math (per batch b):
  y = x @ w_qkv + b_qkv                      # (L, 3D), D=2048, L=2048
  raw reshape (L, 3D) -> (3, NH, L, HD)      # NH=16, HD=128, NO transpose
  => per-head Q/K/V are CONTIGUOUS ranges of y.flatten():
     q_h = flat[(0*NH+h)*L*HD : ...], k_h = flat[(NH+h)*L*HD : ...], ...
  A = softmax((K_h @ Q_h^T)/sqrt(HD), axis=-1);  out_h = A @ V_h
  out_bld[b, i, h*HD:(h+1)*HD] = out_h[i, :];  final = out_bld @ w_out + b_out

Two SPMD launches on 8 cores (the scramble between them runs on host):

Launch A - QKV projection, core k = (batch k//4, column-group k%4):
  y^T chunks [12, 128, L] = (w_qkv col-slice)^T @ x_b^T, in fp8e4
  DoubleRow (256-deep reduction tiles) with a hi+lo residual split at
  matched scales:
      X1=Q(x), X2=Q(x-X1), W1=Q(32w), W2=Q(32w-W1)
      G = X1@W1 + X1@W2 + X2@W1  (one PSUM group; ~bf16 accuracy)
      y' = 16*G = 512*y  (compensated downstream: exp scale /512^2 and
      w_out/512).  0.75x the PE cycles of bf16 at 4x DoubleRow rate.

Launch B - attention + out-proj, core k = (batch, head-group of 4):
  S^T formulation; softmax denominators OFF the PE: exp writes a
  [128, 512, 16] super-tile (j-tile innermost), DVE tensor_reduce sums
  the 16 j-tiles, gpsimd partition_all_reduce sums the partitions
  (replaces the baseline's ones-matmul: -131k PE cycles/core).
  Out-proj (row-parallel partial) interleaved one i-block behind
  attention so the PE stays fed while Act runs exp.  Host sums the 4
  partials per batch and adds b_out.
"""

from contextlib import ExitStack

import numpy as np
import ml_dtypes

import concourse.bass as bass
from concourse import bacc
import concourse.mybir as mybir
import concourse.tile as tile
from concourse.bass_utils import run_bass_kernel_spmd
from concourse.bass_isa import ReduceOp
from concourse.alu_op_type import AluOpType

B, L, D = 2, 2048, 2048
NH, HD = 16, 128
HPC = 4                         # heads per core (launch B)
CPC = 12                        # y^T chunks per core (launch A)
KT = D // 128                   # 16 contraction k-tiles
SCALE = 1.0 / float(np.sqrt(HD))
YS = 512.0                      # y' = YS * y leaves launch A
CEXP = SCALE / (YS * YS)        # exp scale on raw score PSUM

F8 = mybir.dt.float8e4
BF = mybir.dt.bfloat16
F32 = mybir.dt.float32
NP_F8 = ml_dtypes.float8_e4m3fn
NP_BF = ml_dtypes.bfloat16
DR = mybir.MatmulPerfMode.DoubleRow

_CACHE = {}


def _build_a():
    """Core k=(b, cg): y'^T chunks [CPC, 128, L] in bf16, y' = 512*y."""
    nc = bacc.Bacc()
    x1 = nc.dram_tensor("x1", [128, KT, L], F8, kind="ExternalInput")
    x2 = nc.dram_tensor("x2", [128, KT, L], F8, kind="ExternalInput")
    w1 = nc.dram_tensor("w1", [128, 4, KT, 384], F8, kind="ExternalInput")
    w2 = nc.dram_tensor("w2", [128, 4, KT, 384], F8, kind="ExternalInput")
    yt = nc.dram_tensor("yt", [CPC, 128, L], BF, kind="ExternalOutput")

    with tile.TileContext(nc) as tc, ExitStack() as ctx:
        wp = ctx.enter_context(tc.tile_pool(name="wp", bufs=1))
        xp = ctx.enter_context(tc.tile_pool(name="xp", bufs=2))
        outs = ctx.enter_context(tc.tile_pool(name="outs", bufs=6))
        psg = ctx.enter_context(tc.tile_pool(name="psg", bufs=4, space="PSUM"))

        w1_sb = wp.tile([128, 4, KT, 384], F8, tag="w1")
        w2_sb = wp.tile([128, 4, KT, 384], F8, tag="w2")

        # PE warmup during the DMA lead so real matmuls start at full clock
        wu = wp.tile([128, 512], BF, tag="wu")
        nc.vector.memset(wu[:], 0.0)
        for _ in range(28):
            pwu = psg.tile([128, 512], F32, tag="g")
            nc.tensor.matmul(pwu[:], wu[:, 0:128], wu[:], start=True,
                             stop=True)

        # piecewise strip-ordered loads on the DVE queue (outputs go on
        # SP; separate queues avoid head-of-line blocking)
        nc.scalar.dma_start(w1_sb[:, 0], w1[:, 0])
        x1s0 = xp.tile([128, KT, 512], F8, tag="x1s", name="x1s0")
        x2s0 = xp.tile([128, KT, 512], F8, tag="x2s", name="x2s0")
        nc.scalar.dma_start(x1s0[:], x1[:, :, 0:512])
        nc.scalar.dma_start(x2s0[:], x2[:, :, 0:512])
        nc.scalar.dma_start(w2_sb[:, 0], w2[:, 0])
        for pc in range(1, 4):
            nc.scalar.dma_start(w1_sb[:, pc], w1[:, pc])
            nc.scalar.dma_start(w2_sb[:, pc], w2[:, pc])

        for rb in range(4):
            s0, s1 = rb * 512, (rb + 1) * 512
            if rb == 0:
                x1s, x2s = x1s0, x2s0
            else:
                x1s = xp.tile([128, KT, 512], F8, tag="x1s", name=f"x1s{rb}")
                x2s = xp.tile([128, KT, 512], F8, tag="x2s", name=f"x2s{rb}")
                nc.scalar.dma_start(x1s[:], x1[:, :, s0:s1])
                nc.scalar.dma_start(x2s[:], x2[:, :, s0:s1])
            for c in range(CPC):
                g = psg.tile([128, 512], F32, tag="g")
                pc, cc = c // 3, c % 3
                ccs = slice(cc * 128, (cc + 1) * 128)
                for term, (wsb, xsb) in enumerate(
                        ((w1_sb, x1s), (w1_sb, x2s), (w2_sb, x1s))):
                    for kp in range(8):
                        nc.tensor.matmul(
                            g[:],
                            wsb[:, pc, 2 * kp:2 * kp + 2, ccs],
                            xsb[:, 2 * kp:2 * kp + 2, :],
                            start=(term == 0 and kp == 0),
                            stop=(term == 2 and kp == 7),
                            perf_mode=DR,
                        )
                ot = outs.tile([128, 512], BF, tag="o")
                nc.scalar.mul(ot[:], g[:], 16.0)
                nc.sync.dma_start(yt[c, :, s0:s1], ot[:])
    nc.compile()
    return nc


def _build_b():
    """Core (b, g): attention for 4 heads + row-parallel out-proj partial."""
    nc = bacc.Bacc()
    qt = nc.dram_tensor("qt", [128, HPC, L], BF, kind="ExternalInput")
    kt = nc.dram_tensor("kt", [128, HPC, L], BF, kind="ExternalInput")
    v = nc.dram_tensor("v", [128, HPC, L // 128, HD], BF, kind="ExternalInput")
    wo = nc.dram_tensor("wo", [128, HPC, D], BF, kind="ExternalInput")
    fp = nc.dram_tensor("fp", [L, D], F32, kind="ExternalOutput")

    with tile.TileContext(nc) as tc, ExitStack() as ctx:
        singles = ctx.enter_context(tc.tile_pool(name="singles", bufs=1))
        pts = ctx.enter_context(tc.tile_pool(name="pts", bufs=2))
        nrm = ctx.enter_context(tc.tile_pool(name="nrm", bufs=2))
        fout = ctx.enter_context(tc.tile_pool(name="fout", bufs=4))
        psx = ctx.enter_context(tc.tile_pool(name="psx", bufs=2, space="PSUM"))
        pso = ctx.enter_context(tc.tile_pool(name="pso", bufs=2, space="PSUM"))
        psf = ctx.enter_context(tc.tile_pool(name="psf", bufs=2, space="PSUM"))

        qt_sb = singles.tile([128, HPC, L], BF, tag="qt")
        kt_sb = singles.tile([128, HPC, L], BF, tag="kt")
        v_sb = singles.tile([128, HPC, L // 128, HD], BF, tag="v")
        wo_sb = singles.tile([128, HPC, D], BF, tag="wo")
        outT = singles.tile([128, HPC, L], BF, tag="outT")

        # PE warmup during the DMA lead so real matmuls start at full clock
        wu = singles.tile([128, 512], BF, tag="wu")
        nc.vector.memset(wu[:], 0.0)
        for _ in range(14):
            pwu = psx.tile([128, 2, 512], F32, tag="x")
            nc.tensor.matmul(pwu[:, 0, :], wu[:, 0:128], wu[:], start=True,
                             stop=True)

        # per-head loads so head 0's attention starts early; kt sliced so
        # the first score matmul only waits for qt[h0] + kt[h0] slice 0
        for hh in range(HPC):
            nc.sync.dma_start(qt_sb[:, hh, :], qt[:, hh, :])
            for sl in range(4):
                nc.sync.dma_start(kt_sb[:, hh, sl * 512:(sl + 1) * 512],
                                  kt[:, hh, sl * 512:(sl + 1) * 512])
            nc.sync.dma_start(v_sb[:, hh, :, :], v[:, hh, :, :])
        nc.sync.dma_start(wo_sb[:], wo[:])

        def outproj_piece(ib, rb):
            r0 = (ib * 4 + rb) * 128
            for cb in range(4):
                pf = psf.tile([128, 512], F32, tag="f")
                c0 = cb * 512
                for hh in range(HPC):
                    nc.tensor.matmul(
                        pf[:],
                        outT[:, hh, r0:r0 + 128],
                        wo_sb[:, hh, c0:c0 + 512],
                        start=(hh == 0), stop=(hh == HPC - 1),
                    )
                fo = fout.tile([128, 512], F32, tag="fo")
                if cb == 3:
                    nc.scalar.copy(fo[:], pf[:])
                else:
                    nc.vector.tensor_copy(fo[:], pf[:])
                nc.sync.dma_start(fp[r0:r0 + 128, c0:c0 + 512], fo[:])

        for ib in range(4):
            i0, i1 = ib * 512, (ib + 1) * 512
            for hh in range(HPC):
                ptile = pts.tile([128, 16, 512], BF, tag="pt")
                ps_o = pso.tile([128, 512], F32, tag="o")
                sums = nrm.tile([128, 512], BF, tag="s")
                with nc.allow_low_precision(
                        reason="softmax denom j-tile partial sums in bf16; "
                        "fp32 partition allreduce follows"):
                    for jb2 in range(8):
                        ps_x = psx.tile([128, 2, 512], F32, tag="x")
                        for t in range(2):
                            jb = 2 * jb2 + t
                            nc.tensor.matmul(
                                ps_x[:, t, :],
                                qt_sb[:, hh, jb * 128:(jb + 1) * 128],
                                kt_sb[:, hh, i0:i1],
                                start=True, stop=True,
                            )
                        nc.scalar.activation(
                            ptile[:, 2 * jb2:2 * jb2 + 2, :], ps_x[:],
                            mybir.ActivationFunctionType.Exp, scale=CEXP)
                        # j-denominator accumulation rides the exp pipeline
                        if jb2 == 0:
                            nc.vector.tensor_add(
                                sums[:], ptile[:, 0, :], ptile[:, 1, :])
                        else:
                            for t in range(2):
                                nc.vector.tensor_add(
                                    sums[:], sums[:],
                                    ptile[:, 2 * jb2 + t, :])
                for jb in range(16):
                    nc.tensor.matmul(
                        ps_o[:],
                        v_sb[:, hh, jb, :],
                        ptile[:, jb, :],
                        start=(jb == 0), stop=(jb == 15),
                    )
                sumf = nrm.tile([128, 512], F32, tag="sf")
                nc.gpsimd.partition_all_reduce(
                    sumf[:], sums[:], 128, ReduceOp.add)
                recip = nrm.tile([128, 512], F32, tag="r")
                nc.vector.reciprocal(recip[:], sumf[:])
                nc.vector.tensor_mul(
                    out=outT[:, hh, i0:i1], in0=ps_o[:], in1=recip[:])
                if ib > 0:
                    outproj_piece(ib - 1, hh)
        for rb in range(4):
            outproj_piece(3, rb)
    nc.compile()
    return nc


def _get(name):
    if name not in _CACHE:
        _CACHE[name] = _build_a() if name == "a" else _build_b()
    return _CACHE[name]


def _q8(a):
    return a.astype(NP_F8)


def _prep_a(x, w_qkv):
    """Per-core launch-A inputs; core k = (b, cg)."""
    ins = []
    xq = {}
    for b in range(B):
        xt = np.ascontiguousarray(
            x[b].T.reshape(KT, 128, L).transpose(1, 0, 2))
        x1 = _q8(xt)
        x2 = _q8(xt - x1.astype(np.float32))
        xq[b] = (x1, x2)
    for k in range(8):
        b, cg = k // 4, k % 4
        wsl = w_qkv[:, cg * CPC * 128:(cg + 1) * CPC * 128] * 32.0
        wt = np.ascontiguousarray(
            wsl.reshape(KT, 128, CPC * 128).transpose(1, 0, 2))
        wt = np.ascontiguousarray(
            wt.reshape(128, KT, 4, 384).transpose(0, 2, 1, 3))
        w1 = _q8(wt)
        w2 = _q8(wt - w1.astype(np.float32))
        ins.append({"x1": xq[b][0], "x2": xq[b][1], "w1": w1, "w2": w2})
    return ins


def _prep_b(ya_list, b_qkv, w_out):
    """ya_list: 8 arrays [CPC, 128, L] (y' = 512*y); per-core B inputs."""
    sec = L * HD
    ins = []
    for b in range(B):
        yb = np.concatenate([ya_list[b * 4 + cg] for cg in range(4)], axis=0)
        if b_qkv.any():
            yb = (yb.astype(np.float32)
                  + YS * b_qkv.reshape(48, 128)[:, :, None]).astype(NP_BF)
        flat = np.ascontiguousarray(yb.transpose(2, 0, 1)).reshape(-1)
        for g in range(4):
            qts, kts, vs = [], [], []
            for hh in range(HPC):
                h = HPC * g + hh
                qh = flat[h * sec:(h + 1) * sec].reshape(L, HD)
                kh = flat[(NH + h) * sec:(NH + h + 1) * sec].reshape(L, HD)
                vh = flat[(2 * NH + h) * sec:(2 * NH + h + 1) * sec].reshape(
                    L, HD)
                qts.append(qh.T)
                kts.append(kh.T)
                vs.append(vh.reshape(L // 128, 128, HD).transpose(1, 0, 2))
            wsl = w_out[512 * g:512 * (g + 1), :]
            wo_h = np.ascontiguousarray(
                wsl.reshape(HPC, 128, D).transpose(1, 0, 2) / YS).astype(NP_BF)
            ins.append({
                "qt": np.ascontiguousarray(np.stack(qts, axis=1)),
                "kt": np.ascontiguousarray(np.stack(kts, axis=1)),
                "v": np.ascontiguousarray(np.stack(vs, axis=1)),
                "wo": wo_h,
            })
    return ins


def kernel(x, w_qkv, b_qkv, w_out, b_out, _timing=None):
    x = np.asarray(x, dtype=np.float32)
    w_qkv = np.asarray(w_qkv, dtype=np.float32)
    b_qkv = np.asarray(b_qkv, dtype=np.float32)
    w_out = np.asarray(w_out, dtype=np.float32)
    b_out = np.asarray(b_out, dtype=np.float32)
    cores = list(range(8))

    res_a = run_bass_kernel_spmd(_get("a"), _prep_a(x, w_qkv), cores)
    ya = [np.asarray(res_a.results[k]["yt"]) for k in range(8)]

    res_b = run_bass_kernel_spmd(_get("b"), _prep_b(ya, b_qkv, w_out), cores)

    out = np.empty((B, L, D), dtype=np.float32)
    for b in range(B):
        acc = np.zeros((L, D), dtype=np.float32)
        for g in range(4):
            acc += np.asarray(res_b.results[b * 4 + g]["fp"])
        out[b] = acc + b_out[None, :]
    return out


# revision 10
# speedup vs baseline: 1.3794x; 1.0178x over previous
"""Fused attention block (nn_Attention_27865747817251) on 8 trn2 NeuronCores.

Reference math (per batch b):
  y = x @ w_qkv + b_qkv                      # (L, 3D), D=2048, L=2048
  raw reshape (L, 3D) -> (3, NH, L, HD)      # NH=16, HD=128, NO transpose
  => per-head Q/K/V are CONTIGUOUS ranges of y.flatten():
     q_h = flat[(0*NH+h)*L*HD : ...], k_h = flat[(NH+h)*L*HD : ...], ...
  A = softmax((K_h @ Q_h^T)/sqrt(HD), axis=-1);  out_h = A @ V_h
  out_bld[b, i, h*HD:(h+1)*HD] = out_h[i, :];  final = out_bld @ w_out + b_out

Two SPMD launches on 8 cores (the scramble between them runs on host):

Launch A - QKV projection, core k = (batch k//4, column-group k%4):
  y^T chunks [12, 128, L] = (w_qkv col-slice)^T @ x_b^T, in fp8e4
  DoubleRow (256-deep reduction tiles) with a hi+lo residual split at
  matched scales:
      X1=Q(x), X2=Q(x-X1), W1=Q(32w), W2=Q(32w-W1)
      G = X1@W1 + X1@W2 + X2@W1  (one PSUM group; ~bf16 accuracy)
      y' = 16*G = 512*y  (compensated downstream: exp scale /512^2 and
      w_out/512).  0.75x the PE cycles of bf16 at 4x DoubleRow rate.

Launch B - attention + out-proj, core k = (batch, head-group of 4):
  S^T formulation; softmax denominators OFF the PE: exp writes a
  [128, 512, 16] super-tile (j-tile innermost), DVE tensor_reduce sums
  the 16 j-tiles, gpsimd partition_all_reduce sums the partitions
  (replaces the baseline's ones-matmul: -131k PE cycles/core).
  Out-proj (row-parallel partial) interleaved one i-block behind
  attention so the PE stays fed while Act runs exp.  Host sums the 4
  partials per batch and adds b_out.
"""

from contextlib import ExitStack

import numpy as np
import ml_dtypes

import concourse.bass as bass
from concourse import bacc
import concourse.mybir as mybir
import concourse.tile as tile
from concourse.bass_utils import run_bass_kernel_spmd
from concourse.bass_isa import ReduceOp
from concourse.alu_op_type import AluOpType

B, L, D = 2, 2048, 2048
NH, HD = 16, 128
HPC = 4                         # heads per core (launch B)
CPC = 12                        # y^T chunks per core (launch A)
KT = D // 128                   # 16 contraction k-tiles
SCALE = 1.0 / float(np.sqrt(HD))
YS = 512.0                      # y' = YS * y leaves launch A
CEXP = SCALE / (YS * YS)        # exp scale on raw score PSUM

F8 = mybir.dt.float8e4
BF = mybir.dt.bfloat16
F32 = mybir.dt.float32
NP_F8 = ml_dtypes.float8_e4m3fn
NP_BF = ml_dtypes.bfloat16
DR = mybir.MatmulPerfMode.DoubleRow

_CACHE = {}


def _build_a():
    """Core k=(b, cg): y'^T chunks [CPC, 128, L] in bf16, y' = 512*y."""
    nc = bacc.Bacc()
    x1 = nc.dram_tensor("x1", [128, KT, L], F8, kind="ExternalInput")
    x2 = nc.dram_tensor("x2", [128, KT, L], F8, kind="ExternalInput")
    w1 = nc.dram_tensor("w1", [128, CPC, KT, 128], F8, kind="ExternalInput")
    w2 = nc.dram_tensor("w2", [128, CPC, KT, 128], F8, kind="ExternalInput")
    yt = nc.dram_tensor("yt", [CPC, 128, L], BF, kind="ExternalOutput")

    with tile.TileContext(nc) as tc, ExitStack() as ctx:
        wp = ctx.enter_context(tc.tile_pool(name="wp", bufs=1))
        xp = ctx.enter_context(tc.tile_pool(name="xp", bufs=2))
        outs = ctx.enter_context(tc.tile_pool(name="outs", bufs=6))
        psg = ctx.enter_context(tc.tile_pool(name="psg", bufs=4, space="PSUM"))

        w1_sb = wp.tile([128, CPC, KT, 128], F8, tag="w1")
        w2_sb = wp.tile([128, CPC, KT, 128], F8, tag="w2")

        # PE warmup during the DMA lead so real matmuls start at full clock
        wu = wp.tile([128, 512], BF, tag="wu")
        nc.vector.memset(wu[:], 0.0)
        for _ in range(12):
            pwu = psg.tile([128, 512], F32, tag="g")
            nc.tensor.matmul(pwu[:], wu[:, 0:128], wu[:], start=True,
                             stop=True)

        # chunk-granular strip-ordered loads on the Act queue (outputs go
        # on SP; separate queues avoid head-of-line blocking).  The first
        # matmul only needs w1 chunk 0 + x1 strip 0.
        nc.scalar.dma_start(w1_sb[:, 0], w1[:, 0])
        x1s0 = xp.tile([128, KT, 512], F8, tag="x1s", name="x1s0")
        x2s0 = xp.tile([128, KT, 512], F8, tag="x2s", name="x2s0")
        nc.scalar.dma_start(x1s0[:], x1[:, :, 0:512])
        nc.scalar.dma_start(w2_sb[:, 0], w2[:, 0])
        nc.scalar.dma_start(x2s0[:], x2[:, :, 0:512])
        for c in range(1, CPC):
            nc.scalar.dma_start(w1_sb[:, c], w1[:, c])
            nc.scalar.dma_start(w2_sb[:, c], w2[:, c])

        for rb in range(4):
            s0, s1 = rb * 512, (rb + 1) * 512
            if rb == 0:
                x1s, x2s = x1s0, x2s0
            else:
                x1s = xp.tile([128, KT, 512], F8, tag="x1s", name=f"x1s{rb}")
                x2s = xp.tile([128, KT, 512], F8, tag="x2s", name=f"x2s{rb}")
                nc.scalar.dma_start(x1s[:], x1[:, :, s0:s1])
                nc.scalar.dma_start(x2s[:], x2[:, :, s0:s1])
            for c in range(CPC):
                g = psg.tile([128, 512], F32, tag="g")
                for term, (wsb, xsb) in enumerate(
                        ((w1_sb, x1s), (w2_sb, x1s), (w1_sb, x2s))):
                    for kp in range(8):
                        nc.tensor.matmul(
                            g[:],
                            wsb[:, c, 2 * kp:2 * kp + 2, :],
                            xsb[:, 2 * kp:2 * kp + 2, :],
                            start=(term == 0 and kp == 0),
                            stop=(term == 2 and kp == 7),
                            perf_mode=DR,
                        )
                ot = outs.tile([128, 512], BF, tag="o")
                nc.scalar.mul(ot[:], g[:], 16.0)
                nc.sync.dma_start(yt[c, :, s0:s1], ot[:])
    nc.compile()
    return nc


def _build_b():
    """Core (b, g): attention for 4 heads + row-parallel out-proj partial."""
    nc = bacc.Bacc()
    qt = nc.dram_tensor("qt", [128, HPC, L], BF, kind="ExternalInput")
    kt = nc.dram_tensor("kt", [128, HPC, L], BF, kind="ExternalInput")
    v = nc.dram_tensor("v", [128, HPC, L // 128, HD], BF, kind="ExternalInput")
    wo = nc.dram_tensor("wo", [128, HPC, D], BF, kind="ExternalInput")
    fp = nc.dram_tensor("fp", [L, D], F32, kind="ExternalOutput")

    with tile.TileContext(nc) as tc, ExitStack() as ctx:
        singles = ctx.enter_context(tc.tile_pool(name="singles", bufs=1))
        pts = ctx.enter_context(tc.tile_pool(name="pts", bufs=2))
        nrm = ctx.enter_context(tc.tile_pool(name="nrm", bufs=2))
        fout = ctx.enter_context(tc.tile_pool(name="fout", bufs=4))
        psx = ctx.enter_context(tc.tile_pool(name="psx", bufs=2, space="PSUM"))
        pso = ctx.enter_context(tc.tile_pool(name="pso", bufs=2, space="PSUM"))
        psf = ctx.enter_context(tc.tile_pool(name="psf", bufs=2, space="PSUM"))

        qt_sb = singles.tile([128, HPC, L], BF, tag="qt")
        kt_sb = singles.tile([128, HPC, L], BF, tag="kt")
        v_sb = singles.tile([128, HPC, L // 128, HD], BF, tag="v")
        wo_sb = singles.tile([128, HPC, D], BF, tag="wo")
        outT = singles.tile([128, HPC, L], BF, tag="outT")

        # PE warmup during the DMA lead so real matmuls start at full clock
        wu = singles.tile([128, 512], BF, tag="wu")
        nc.vector.memset(wu[:], 0.0)
        for _ in range(12):
            pwu = psx.tile([128, 2, 512], F32, tag="x")
            nc.tensor.matmul(pwu[:, 0, :], wu[:, 0:128], wu[:], start=True,
                             stop=True)

        # per-head loads so head 0's attention starts early; kt sliced so
        # the first score matmul only waits for qt[h0] + kt[h0] slice 0
        for hh in range(HPC):
            if hh == 0:
                nc.sync.dma_start(qt_sb[:, 0, 0:1024], qt[:, 0, 0:1024])
                nc.sync.dma_start(qt_sb[:, 0, 1024:2048], qt[:, 0, 1024:2048])
            else:
                nc.sync.dma_start(qt_sb[:, hh, :], qt[:, hh, :])
            for sl in range(4):
                nc.sync.dma_start(kt_sb[:, hh, sl * 512:(sl + 1) * 512],
                                  kt[:, hh, sl * 512:(sl + 1) * 512])
            nc.sync.dma_start(v_sb[:, hh, :, :], v[:, hh, :, :])
        nc.sync.dma_start(wo_sb[:], wo[:])

        def outproj_piece(ib, rb):
            r0 = (ib * 4 + rb) * 128
            for cb in range(4):
                pf = psf.tile([128, 512], F32, tag="f")
                c0 = cb * 512
                for hh in range(HPC):
                    nc.tensor.matmul(
                        pf[:],
                        outT[:, hh, r0:r0 + 128],
                        wo_sb[:, hh, c0:c0 + 512],
                        start=(hh == 0), stop=(hh == HPC - 1),
                    )
                fo = fout.tile([128, 512], F32, tag="fo")
                if cb % 2 == 1:
                    nc.scalar.copy(fo[:], pf[:])
                else:
                    nc.vector.tensor_copy(fo[:], pf[:])
                nc.sync.dma_start(fp[r0:r0 + 128, c0:c0 + 512], fo[:])

        for ib in range(4):
            i0, i1 = ib * 512, (ib + 1) * 512
            for hh in range(HPC):
                ptile = pts.tile([128, 16, 512], BF, tag="pt")
                ps_o = pso.tile([128, 512], F32, tag="o")
                sums = nrm.tile([128, 512], BF, tag="s")
                with nc.allow_low_precision(
                        reason="softmax denom j-tile partial sums in bf16; "
                        "fp32 partition allreduce follows"):
                    for jb2 in range(8):
                        ps_x = psx.tile([128, 2, 512], F32, tag="x")
                        for t in range(2):
                            jb = 2 * jb2 + t
                            nc.tensor.matmul(
                                ps_x[:, t, :],
                                qt_sb[:, hh, jb * 128:(jb + 1) * 128],
                                kt_sb[:, hh, i0:i1],
                                start=True, stop=True,
                            )
                        nc.scalar.activation(
                            ptile[:, 2 * jb2:2 * jb2 + 2, :], ps_x[:],
                            mybir.ActivationFunctionType.Exp, scale=CEXP)
                        # j-denominator accumulation rides the exp pipeline
                        if jb2 == 0:
                            nc.vector.tensor_add(
                                sums[:], ptile[:, 0, :], ptile[:, 1, :])
                        else:
                            for t in range(2):
                                nc.vector.tensor_add(
                                    sums[:], sums[:],
                                    ptile[:, 2 * jb2 + t, :])
                for jb in range(16):
                    nc.tensor.matmul(
                        ps_o[:],
                        v_sb[:, hh, jb, :],
                        ptile[:, jb, :],
                        start=(jb == 0), stop=(jb == 15),
                    )
                sumf = nrm.tile([128, 512], F32, tag="sf")
                nc.gpsimd.partition_all_reduce(
                    sumf[:], sums[:], 128, ReduceOp.add)
                recip = nrm.tile([128, 512], F32, tag="r")
                nc.vector.reciprocal(recip[:], sumf[:])
                nc.vector.tensor_mul(
                    out=outT[:, hh, i0:i1], in0=ps_o[:], in1=recip[:])
                if ib > 0:
                    outproj_piece(ib - 1, hh)
        for rb in range(4):
            outproj_piece(3, rb)
    nc.compile()
    return nc


def _get(name):
    if name not in _CACHE:
        _CACHE[name] = _build_a() if name == "a" else _build_b()
    return _CACHE[name]


def _q8(a):
    return a.astype(NP_F8)


def _prep_a(x, w_qkv):
    """Per-core launch-A inputs; core k = (b, cg)."""
    ins = []
    xq = {}
    for b in range(B):
        xt = np.ascontiguousarray(
            x[b].T.reshape(KT, 128, L).transpose(1, 0, 2))
        x1 = _q8(xt)
        x2 = _q8(xt - x1.astype(np.float32))
        xq[b] = (x1, x2)
    for k in range(8):
        b, cg = k // 4, k % 4
        wsl = w_qkv[:, cg * CPC * 128:(cg + 1) * CPC * 128] * 32.0
        wt = np.ascontiguousarray(
            wsl.reshape(KT, 128, CPC * 128).transpose(1, 0, 2))
        wt = np.ascontiguousarray(
            wt.reshape(128, KT, CPC, 128).transpose(0, 2, 1, 3))
        w1 = _q8(wt)
        w2 = _q8(wt - w1.astype(np.float32))
        ins.append({"x1": xq[b][0], "x2": xq[b][1], "w1": w1, "w2": w2})
    return ins


def _prep_b(ya_list, b_qkv, w_out):
    """ya_list: 8 arrays [CPC, 128, L] (y' = 512*y); per-core B inputs."""
    sec = L * HD
    ins = []
    for b in range(B):
        yb = np.concatenate([ya_list[b * 4 + cg] for cg in range(4)], axis=0)
        if b_qkv.any():
            yb = (yb.astype(np.float32)
                  + YS * b_qkv.reshape(48, 128)[:, :, None]).astype(NP_BF)
        flat = np.ascontiguousarray(yb.transpose(2, 0, 1)).reshape(-1)
        for g in range(4):
            qts, kts, vs = [], [], []
            for hh in range(HPC):
                h = HPC * g + hh
                qh = flat[h * sec:(h + 1) * sec].reshape(L, HD)
                kh = flat[(NH + h) * sec:(NH + h + 1) * sec].reshape(L, HD)
                vh = flat[(2 * NH + h) * sec:(2 * NH + h + 1) * sec].reshape(
                    L, HD)
                qts.append(qh.T)
                kts.append(kh.T)
                vs.append(vh.reshape(L // 128, 128, HD).transpose(1, 0, 2))
            wsl = w_out[512 * g:512 * (g + 1), :]
            wo_h = np.ascontiguousarray(
                wsl.reshape(HPC, 128, D).transpose(1, 0, 2) / YS).astype(NP_BF)
            ins.append({
                "qt": np.ascontiguousarray(np.stack(qts, axis=1)),
                "kt": np.ascontiguousarray(np.stack(kts, axis=1)),
                "v": np.ascontiguousarray(np.stack(vs, axis=1)),
                "wo": wo_h,
            })
    return ins


def kernel(x, w_qkv, b_qkv, w_out, b_out, _timing=None):
    x = np.asarray(x, dtype=np.float32)
    w_qkv = np.asarray(w_qkv, dtype=np.float32)
    b_qkv = np.asarray(b_qkv, dtype=np.float32)
    w_out = np.asarray(w_out, dtype=np.float32)
    b_out = np.asarray(b_out, dtype=np.float32)
    cores = list(range(8))

    res_a = run_bass_kernel_spmd(_get("a"), _prep_a(x, w_qkv), cores)
    ya = [np.asarray(res_a.results[k]["yt"]) for k in range(8)]

    res_b = run_bass_kernel_spmd(_get("b"), _prep_b(ya, b_qkv, w_out), cores)

    out = np.empty((B, L, D), dtype=np.float32)
    for b in range(B):
        acc = np.zeros((L, D), dtype=np.float32)
        for g in range(4):
            acc += np.asarray(res_b.results[b * 4 + g]["fp"])
        out[b] = acc + b_out[None, :]
    return out
